# revision 22
# baseline (speedup 1.0000x reference)
"""Separable one-stage spectral kernel for nn_Dynamics_2748779069592 (TRN2, 8 cores).

Out_n = Z0 + n*Y1 + O(n^2 eps^2),  Y1 = Qc[(g16-1) .* W0 + DT*S16 .* Qtil]Qc^T
with g16 = (1 + DT*NU*(lam_i+lam_j))^16.  Because DT*NU is tiny, the
spectral multipliers are separable to first order:
  g16 - 1     ~ ex_i + ex_j          (ex = (1+a*lam)^16 - 1, err ~3e-4 rel)
  DT*S16      ~ 16*DT + 120*a*DT*(lam_i+lam_j)   (err ~2e-7 rel)
which collapses the 4-deep transform sandwich into ONE matmul stage:
  Y1 ~= E z + z E + A Q + Q A
  E = Qc diag((1+a lam)^16 - 1) Qc^T   (entries ~1e-4 -> bf16-safe)
  A = 8*DT*I + 120*a*DT*C              (C = 1D periodic stencil)
Host supplies z^T and Q^T so both one-sided products run as plain matmuls.
Measured 2.5e-3 rel err in fp64 simulation (gate 2e-2).

The device computes delta_n = n*Y1 in bf16; the host adds Z0 back.

Per core (pure data parallel, 2 elems x 16 times): 32 matmuls accumulated
into 4 PSUM groups, evac = delta_1, then 30 scale-copies split
DVE/ACT/Pool, outputs written as 8 grouped 512KB DMAs alternating between
the SP and ACT hardware DGE queues.

Tiles use the [128, 512] swizzled layout (partition p holds grid rows p
and p+128); mm256(A, B) computes A^T B in that layout.
"""
import sys

sys.path.insert(0, "/opt/trn_rl_repo")
import warnings

warnings.filterwarnings("ignore")
import numpy as np
from ml_dtypes import bfloat16

N = 256
P = 128
NE = 2  # elems per core
NT = 16  # output times per core
NG = 4  # output DMA groups per elem (4 times each)
NCORES = 8
DT = 1e-3
NU = 1e-2
W = 2 * N

_compiled = None


def swz(x):
    """[..., 256, 256] -> [..., 128, 512] on-chip layout (rows p, p+128)."""
    sh = x.shape[:-2]
    return (
        np.asarray(x).reshape(sh + (2, P, N)).swapaxes(-3, -2).reshape(sh + (P, 2 * N))
    )


def unswz(t):
    """[..., 128, 512] -> [..., 256, 256]."""
    sh = t.shape[:-2]
    return t.reshape(sh + (P, 2, N)).swapaxes(-3, -2).reshape(sh + (N, N))


def _make_tables():
    C = np.zeros((N, N))
    i = np.arange(N)
    C[i, (i + 1) % N] = 1.0
    C[i, (i - 1) % N] = 1.0
    C[i, i] = -2.0
    lam, Qc = np.linalg.eigh(C)
    a = DT * NU
    E = (Qc * ((1.0 + a * lam) ** 16 - 1.0)) @ Qc.T
    A = 8.0 * DT * np.eye(N) + 120.0 * a * DT * C
    return E, A


def _build():
    import concourse.bacc as bacc
    import concourse.mybir as mybir
    from concourse.tile import TileContext

    bf = mybir.dt.bfloat16
    nc = bacc.Bacc("TRN2", target_bir_lowering=False, debug=False)

    # sync queue: [z0, zT0]; ACT queue: [E], [A, q, qT], [z1, zT1]
    ga_d = nc.dram_tensor("ga", [P, 2 * W], bf, kind="ExternalInput")
    ge_d = nc.dram_tensor("ge", [P, W], bf, kind="ExternalInput")
    gb_d = nc.dram_tensor("gb", [P, 3 * W], bf, kind="ExternalInput")
    gc_d = nc.dram_tensor("gc", [P, 2 * W], bf, kind="ExternalInput")
    out_d = nc.dram_tensor("out", [NE, NG, P, 4 * W], bf, kind="ExternalOutput")

    with TileContext(nc) as tc:
        with (
            tc.tile_pool(name="const", bufs=1) as cpool,
            tc.tile_pool(name="warm", bufs=1) as wmpool,
            tc.tile_pool(name="outp", bufs=8) as opool,
            tc.tile_pool(name="psum", bufs=4, space="PSUM") as psum,
            tc.tile_pool(name="wpsum", bufs=1, space="PSUM") as wpsum,
        ):
            _uid = [0]

            def nm(tag):
                _uid[0] += 1
                return f"{tag}_{_uid[0]}"

            # PE warmup: hold the tensor engine busy through the input-load
            # wait so the p-state is at max when the real matmuls arrive.
            wm_t = wmpool.tile([P, N], bf, tag="wm", name=nm("wm"))
            nc.vector.memset(wm_t[:, :], 0.0)
            wpt = wpsum.tile([P, N], mybir.dt.float32, tag="wps", name=nm("wps"))
            for _ in range(22):
                nc.tensor.matmul(
                    wpt[:, :], wm_t[:, 0:P], wm_t[:, :], start=True, stop=True
                )

            ga_t = cpool.tile([P, 2 * W], bf, tag="ga", name=nm("ga"))
            nc.sync.dma_start(out=ga_t[:, :], in_=ga_d.ap()[:, :])
            ge_t = cpool.tile([P, W], bf, tag="ge", name=nm("ge"))
            nc.scalar.dma_start(out=ge_t[:, :], in_=ge_d.ap()[:, :])
            gb_t = cpool.tile([P, 3 * W], bf, tag="gb", name=nm("gb"))
            nc.scalar.dma_start(out=gb_t[:, :], in_=gb_d.ap()[:, :])
            gc_t = cpool.tile([P, 2 * W], bf, tag="gc", name=nm("gc"))
            nc.scalar.dma_start(out=gc_t[:, :], in_=gc_d.ap()[:, :])

            e_t = ge_t[:, 0:W]
            z_t = [ga_t[:, 0:W], gc_t[:, 0:W]]
            zt_t = [ga_t[:, W : 2 * W], gc_t[:, W : 2 * W]]
            a_t = gb_t[:, 0:W]
            q_t = gb_t[:, W : 2 * W]
            qt_t = gb_t[:, 2 * W : 3 * W]

            # output group tiles; delta_1 = Y1 lands in group 0, slice 0
            grp = [
                [opool.tile([P, 4 * W], bf, tag="o", name=nm("o")) for _ in range(NG)]
                for _ in range(NE)
            ]
            y1 = [grp[e][0][:, 0:W] for e in range(NE)]

            # ---- one-stage accumulation: Y1 = E z + z E + A Q + Q A ----
            for e in range(NE):
                pts = []
                for m in range(2):
                    pt = psum.tile([P, N], mybir.dt.float32, tag="ps", name=nm("ps"))
                    pts.append(pt)
                    first = True
                    for lhs_t, rhs_t in (
                        (e_t, z_t[e]),
                        (zt_t[e], e_t),
                        (a_t, q_t),
                        (qt_t, a_t),
                    ):
                        for k in range(2):
                            nc.tensor.matmul(
                                pt[:, :],
                                lhs_t[:, N * k + P * m : N * k + P * m + P],
                                rhs_t[:, N * k : N * k + N],
                                start=first,
                                stop=(lhs_t is qt_t and k == 1),
                            )
                            first = False
                # evac both halves on DVE (ACT's queue lags its first ops)
                nc.vector.tensor_copy(y1[e][:, 0:N], pts[0][:, :])
                nc.vector.tensor_copy(y1[e][:, N : 2 * N], pts[1][:, :])

            # ---- assembly: delta_n = n * Y1 (n>=2); Pool uses chain-adds.
            # Group-closing slices (n=4,8,12,16) stay on fast DVE so Pool
            # never gates a DMA issue.
            ENG = {
                0: {2: "v", 3: "v", 4: "v", 5: "v", 6: "p", 7: "v", 8: "v",
                    9: "a", 10: "v", 11: "v", 12: "v", 13: "a", 14: "p",
                    15: "v", 16: "v"},
                1: {2: "v", 3: "v", 4: "v", 5: "v", 6: "p", 7: "v", 8: "v",
                    9: "a", 10: "v", 11: "v", 12: "v", 13: "a", 14: "v",
                    15: "p", 16: "v"},
            }
            prev = [y1[0], y1[1]]
            for g in range(NG):
                for e in range(NE):
                    for j in range(4):
                        n = 4 * g + j + 1
                        if n == 1:
                            continue
                        o_sl = grp[e][g][:, j * W : (j + 1) * W]
                        w = ENG[e][n]
                        if w == "a":
                            nc.scalar.mul(o_sl, y1[e][:, :], float(n))
                        elif w == "p":
                            nc.gpsimd.tensor_add(o_sl, prev[e][:, :], y1[e][:, :])
                        else:
                            nc.vector.tensor_scalar_mul(o_sl, y1[e][:, :], float(n))
                        prev[e] = o_sl
                for e in range(NE):
                    if g == NG - 1:
                        dq = nc.gpsimd  # 3rd (software DGE) queue for last group
                    else:
                        dq = nc.sync if e == 0 else nc.scalar
                    dq.dma_start(out=out_d.ap()[e, g], in_=grp[e][g][:, :])

    nc.compile()
    return nc


def _get_compiled():
    global _compiled
    if _compiled is None:
        _compiled = _build()
    return _compiled


def _run(inputs_full, Q, trace=False):
    from concourse import bass_utils

    nc = _get_compiled()
    E, A = _make_tables()
    z32 = np.asarray(inputs_full, np.float32)
    zs = swz(z32).astype(bfloat16)
    zts = swz(z32.swapaxes(-1, -2)).astype(bfloat16)
    Q32 = np.asarray(Q, np.float32)
    es, as_ = swz(E).astype(bfloat16), swz(A).astype(bfloat16)
    qs, qts = swz(Q32).astype(bfloat16), swz(Q32.T).astype(bfloat16)
    ge = np.ascontiguousarray(es)
    gb = np.ascontiguousarray(np.stack([as_, qs, qts], axis=1)).reshape(P, 3 * W)
    in_maps = []
    for c in range(NCORES):
        e0, e1 = NE * c, NE * c + 1
        ga = np.ascontiguousarray(np.stack([zs[e0], zts[e0]], axis=1)).reshape(
            P, 2 * W
        )
        gc = np.ascontiguousarray(np.stack([zs[e1], zts[e1]], axis=1)).reshape(
            P, 2 * W
        )
        in_maps.append({"ga": ga, "ge": ge, "gb": gb, "gc": gc})
    kw = dict(trace=True) if trace else {}
    last_err = None
    for attempt in range(3):
        try:
            res = bass_utils.run_bass_kernel_spmd(
                nc, in_maps, core_ids=list(range(NCORES)), **kw
            )
            break
        except Exception as exc:  # rare transient device error; retry
            last_err = exc
            import time

            time.sleep(5)
    else:
        raise last_err
    out = np.empty((16, 16, N, N), dtype=np.float32)
    for c in range(NCORES):
        r = np.asarray(res.results[c]["out"])  # [NE, NG, P, 4W] bf16
        delta = r.reshape(NE, NG, P, 4, W).transpose(0, 1, 3, 2, 4)
        delta = unswz(delta.reshape(NE, NT, P, W).astype(np.float32))
        out[NE * c : NE * (c + 1)] = delta + z32[NE * c : NE * (c + 1), None]
    return out, res


def kernel(inputs, Q):
    inputs = np.ascontiguousarray(np.asarray(inputs, dtype=np.float32))
    Q = np.ascontiguousarray(np.asarray(Q, dtype=np.float32))
    out, _ = _run(inputs, Q, trace=False)
    return out
